# revision 1
# baseline (speedup 1.0000x reference)
"""Trainium2 Bass kernel for nn_DVGAE (GCN encoder + inner-product decoder).

Strategy (8 NeuronCores, SPMD):
  - Nodes sharded 1250/core. Edges partitioned by destination node.
  - P0: h = l2norm(x @ W2.T + b2) * 1.8 ; xw = h @ Wg.T  (per-core node shard,
    b2 folded in as an extra contraction row). AllGather xw (bf16).
  - GCN aggregation as matmul: edges grouped into 10 dest-tiles of 128 dests,
    chunks of 128 edges; per chunk gather xw[src] rows (dma_gather) and build
    Sel[e,d] = (iota==destloc_e)*norm_e in one fused DVE op; PE accumulates
    Sel.T @ rows into the dest tile = segment sum. Self-loops and a bias
    pseudo-node (xw row = bg, norm 1) are folded into the edge list.
  - z2 = l2norm(x2 @ W22.T)*0.8: stream host-transposed bf16 x2.T through PE
    (contract over 10000, M=2), row norm on-chip.
  - AllGather z1 (128ch) packed with z2[:,0] into 256-col bf16 rows; decode
    gathers both endpoint rows per edge, DVE dot + sigmoids.
"""

import sys

sys.path.insert(0, "/opt/trn_rl_repo")

import numpy as np
import ml_dtypes

import concourse.bass as bass
import concourse.bacc as bacc
import concourse.mybir as mybir
import concourse.tile as tile
from concourse.bass_utils import run_bass_kernel_spmd
from concourse.masks import make_identity

P = 128
NCORES = 8
N = 10000
E = 320000
IN_DIM = 512
C = 128
NS = N // NCORES          # 1250 nodes per core
NTILES = (NS + P - 1) // P  # 10 dest tiles per core (last has 98 dests)
SCALING_FACTOR = 1.8
SC = 0.8
L2_EPS = 1e-12

NEXT = N + 16             # gather tables padded to 10016 rows
BIAS_ROW = N              # xw_ext[10000] = bg
ZERO_ROW = N + 1          # all-zero row (pad edges)

KT = 80                   # k-tiles of 128 over the 10240-padded contraction dim
KB = 4                    # k-tiles per x2 stream DMA (512 rows, ~1.25MB)
NCHUNK_N = 10             # P0 node chunks of 125
NW = NS // NCHUNK_N       # 125

bf16 = mybir.dt.bfloat16
fp32 = mybir.dt.float32
i16 = mybir.dt.int16
i32 = mybir.dt.int32

_F32 = np.float32
_BF16 = ml_dtypes.bfloat16


def _build_program(NT):
    """One SPMD Bass program; NT = chunks per dest tile (compile-time)."""
    NCH = NTILES * NT
    nc = bacc.Bacc(None, target_bir_lowering=False, debug=False)

    # ---------------- I/O ----------------
    xT_aug = nc.declare_dram_parameter("xT_aug", [5 * P, NS], fp32, isOutput=False)
    w2T_aug = nc.declare_dram_parameter("w2T_aug", [5 * P, C], fp32, isOutput=False)
    wgT = nc.declare_dram_parameter("wgT", [C, C], fp32, isOutput=False)
    x2T = nc.declare_dram_parameter("x2T", [KT * P, NS], bf16, isOutput=False)
    w22T = nc.declare_dram_parameter("w22T", [KT * P, 2], bf16, isOutput=False)
    xw_extra = nc.declare_dram_parameter("xw_extra", [16, C], bf16, isOutput=False)
    z1_extra = nc.declare_dram_parameter("z1_extra", [16, 2 * C], bf16, isOutput=False)
    agg_idx = nc.declare_dram_parameter("agg_idx", [P, NCH * 8], i16, isOutput=False)
    dst_idx = nc.declare_dram_parameter("dst_idx", [P, NCH * 8], i16, isOutput=False)
    destloc = nc.declare_dram_parameter("destloc", [P, NCH], fp32, isOutput=False)
    enorm = nc.declare_dram_parameter("enorm", [P, NCH], fp32, isOutput=False)

    dec_out = nc.declare_dram_parameter("dec_out", [P, NCH], fp32, isOutput=True)

    # ------------- internal DRAM -------------
    ag1_in = nc.dram_tensor("ag1_in", [NS, C], bf16)
    xw_ext = nc.dram_tensor("xw_ext", [N, C], bf16, addr_space="Shared")
    xw_gat = nc.dram_tensor("xw_gat", [NEXT, C], bf16)
    ag2_in = nc.dram_tensor("ag2_in", [NS, 132], bf16)
    z1_ext = nc.dram_tensor("z1_ext", [N, 132], bf16, addr_space="Shared")
    z1_gat = nc.dram_tensor("z1_gat", [NEXT, 2 * C], bf16)

    rg = [list(range(NCORES))]

    with tile.TileContext(nc) as tc:
        with (
            tc.tile_pool(name="const", bufs=1) as cpool,
            tc.tile_pool(name="sb", bufs=2) as sb,
            tc.tile_pool(name="selp", bufs=102) as selpool,
            tc.tile_pool(name="x2s", bufs=3) as x2pool,
            tc.tile_pool(name="gx", bufs=2) as gxpool,
            tc.tile_pool(name="dec", bufs=2) as decpool,
            tc.tile_pool(name="acc", bufs=1) as accpool,
            tc.tile_pool(name="ps_p0", bufs=1, space="PSUM") as ps_p0,
            tc.tile_pool(name="ps_tp", bufs=1, space="PSUM") as ps_tp,
            tc.tile_pool(name="ps_xw", bufs=1, space="PSUM") as ps_xw,
            tc.tile_pool(name="ps_z2", bufs=2, space="PSUM") as ps_z2,
            tc.tile_pool(name="ps_z1", bufs=2, space="PSUM") as ps_z1,
        ):
            # ---------- constants ----------
            ident = cpool.tile([P, P], fp32)
            make_identity(nc, ident[:])
            iota_i = cpool.tile([P, P], mybir.dt.int32)
            nc.gpsimd.iota(iota_i[:], pattern=[[1, P]], base=0, channel_multiplier=0)
            iota_f = cpool.tile([P, P], fp32)
            nc.vector.tensor_copy(out=iota_f[:], in_=iota_i[:])

            w2T_sb = cpool.tile([P, 5, C], fp32)
            nc.sync.dma_start(
                out=w2T_sb[:], in_=w2T_aug[:].rearrange("(t p) c -> p t c", p=P)
            )
            wgT_sb = cpool.tile([P, C], fp32)
            nc.sync.dma_start(out=wgT_sb[:], in_=wgT[:])
            w22T_sb = cpool.tile([P, KT, 2], bf16)
            nc.sync.dma_start(
                out=w22T_sb[:], in_=w22T[:].rearrange("(t p) c -> p t c", p=P)
            )
            xT_sb = cpool.tile([P, 5, NS], fp32)
            nc.sync.dma_start(
                out=xT_sb[:], in_=xT_aug[:].rearrange("(t p) n -> p t n", p=P)
            )
            aggidx_sb = cpool.tile([P, NCH * 8], i16)
            nc.sync.dma_start(out=aggidx_sb[:], in_=agg_idx[:])
            dstidx_sb = cpool.tile([P, NCH * 8], i16)
            nc.sync.dma_start(out=dstidx_sb[:], in_=dst_idx[:])
            dloc_sb = cpool.tile([P, NCH], fp32)
            nc.sync.dma_start(out=dloc_sb[:], in_=destloc[:])
            nrm_sb = cpool.tile([P, NCH], fp32)
            nc.sync.dma_start(out=nrm_sb[:], in_=enorm[:])

            # zero-fill the decode gather table (pad cols beyond 132)
            zfill = cpool.tile([P, NEXT * 2 * C // (2 * P)], bf16)
            nc.vector.memset(zfill[:], 0.0)
            nc.gpsimd.dma_start(out=z1_gat[0 : NEXT // 2, :], in_=zfill[:])
            nc.gpsimd.dma_start(out=z1_gat[NEXT // 2 : NEXT, :], in_=zfill[:])

            # extra rows of the gather tables
            xw_extra_sb = sb.tile([16, C], bf16, tag="extra")
            nc.sync.dma_start(out=xw_extra_sb[:], in_=xw_extra[:])
            nc.gpsimd.dma_start(out=xw_gat[N:NEXT, :], in_=xw_extra_sb[:])
            z1_extra_sb = sb.tile([16, 2 * C], bf16, tag="extra2")
            nc.sync.dma_start(out=z1_extra_sb[:], in_=z1_extra[:])
            nc.gpsimd.dma_start(out=z1_gat[N:NEXT, :], in_=z1_extra_sb[:])

            # ---------- P0: h = l2norm(x@W2.T + b2)*1.8 ; xw = h@Wg.T ----------
            for nb in range(NCHUNK_N):
                n0 = nb * NW
                h_ps = ps_p0.tile([NW, C], fp32, space="PSUM", tag="h")
                for t in range(5):
                    nc.tensor.matmul(
                        out=h_ps[:],
                        lhsT=xT_sb[:, t, n0 : n0 + NW],
                        rhs=w2T_sb[:, t, :],
                        start=(t == 0),
                        stop=(t == 4),
                    )
                sq = sb.tile([NW, C], fp32, tag="sq")
                ss = sb.tile([NW, 1], fp32, tag="ss")
                nc.scalar.activation(
                    out=sq[:], in_=h_ps[:],
                    func=mybir.ActivationFunctionType.Square,
                    accum_out=ss[:, :1],
                )
                sroot = sb.tile([NW, 1], fp32, tag="sroot")
                nc.scalar.activation(
                    out=sroot[:, :1], in_=ss[:, :1],
                    func=mybir.ActivationFunctionType.Sqrt,
                )
                nc.vector.tensor_scalar_max(sroot[:, :1], sroot[:, :1], L2_EPS)
                rinv = sb.tile([NW, 1], fp32, tag="rinv")
                nc.vector.reciprocal(rinv[:, :1], sroot[:, :1])
                nc.scalar.activation(
                    out=rinv[:, :1], in_=rinv[:, :1],
                    func=mybir.ActivationFunctionType.Copy, scale=SCALING_FACTOR,
                )
                h2 = sb.tile([NW, C], fp32, tag="h2")
                nc.scalar.activation(
                    out=h2[:], in_=h_ps[:],
                    func=mybir.ActivationFunctionType.Copy, scale=rinv[:, :1],
                )
                h2T_ps = ps_tp.tile([C, NW], fp32, space="PSUM", tag="tp")
                nc.tensor.matmul(
                    out=h2T_ps[:], lhsT=h2[:], rhs=ident[:NW, :NW], is_transpose=True
                )
                h2T = sb.tile([C, NW], fp32, tag="h2T")
                nc.vector.tensor_copy(out=h2T[:], in_=h2T_ps[:])
                xw_ps = ps_xw.tile([NW, C], fp32, space="PSUM", tag="xw")
                nc.tensor.matmul(
                    out=xw_ps[:], lhsT=h2T[:], rhs=wgT_sb[:], start=True, stop=True
                )
                xw_bf = sb.tile([NW, C], bf16, tag="xwbf")
                nc.vector.tensor_copy(out=xw_bf[:], in_=xw_ps[:])
                nc.gpsimd.dma_start(out=ag1_in[n0 : n0 + NW, :], in_=xw_bf[:])

            nc.gpsimd.collective_compute(
                "AllGather",
                mybir.AluOpType.bypass,
                ins=[ag1_in[:]],
                outs=[xw_ext[:]],
                replica_groups=rg,
            )
            nc.gpsimd.dma_start(out=xw_gat[0:N, :], in_=xw_ext[:])

            # ---------- P5: z2 = l2norm(x2 @ W22.T) * 0.8 ----------
            z2v = accpool.tile([2, NS], fp32, tag="z2v")
            FCH = [(0, 512), (512, 512), (1024, NS - 1024)]
            for f0, fw in FCH:
                z2_ps = ps_z2.tile([2, 512], fp32, space="PSUM", tag="z2")
                for b in range(KT // KB):
                    xt = x2pool.tile([P, KB, 512], bf16, tag="x2t")
                    nc.sync.dma_start(
                        out=xt[:, :, :fw],
                        in_=x2T[b * KB * P : (b + 1) * KB * P, f0 : f0 + fw].rearrange(
                            "(a p) n -> p a n", p=P
                        ),
                    )
                    for a in range(KB):
                        kt = b * KB + a
                        nc.tensor.matmul(
                            out=z2_ps[:, :fw],
                            lhsT=w22T_sb[:, kt, :],
                            rhs=xt[:, a, :fw],
                            start=(kt == 0),
                            stop=(kt == KT - 1),
                        )
                nc.vector.tensor_copy(out=z2v[:, f0 : f0 + fw], in_=z2_ps[:, :fw])

            # per dest-tile: transpose [2, dt] -> [dt, 2], row-normalize, keep col 0
            z2col = accpool.tile([P, NTILES], fp32, tag="z2col")
            for t in range(NTILES):
                dt = min(P, NS - t * P)
                z2t_ps = ps_tp.tile([P, 2], fp32, space="PSUM", tag="tp2")
                nc.tensor.matmul(
                    out=z2t_ps[:dt, :],
                    lhsT=z2v[:, t * P : t * P + dt],
                    rhs=ident[0:2, 0:2],
                    is_transpose=True,
                )
                z2t = sb.tile([P, 2], fp32, tag="z2t")
                nc.vector.tensor_copy(out=z2t[:dt, :], in_=z2t_ps[:dt, :])
                z2sq = sb.tile([P, 2], fp32, tag="z2sq")
                z2ss = sb.tile([P, 1], fp32, tag="z2ss")
                nc.scalar.activation(
                    out=z2sq[:dt, :], in_=z2t[:dt, :],
                    func=mybir.ActivationFunctionType.Square,
                    accum_out=z2ss[:dt, :1],
                )
                nc.scalar.activation(
                    out=z2ss[:dt, :1], in_=z2ss[:dt, :1],
                    func=mybir.ActivationFunctionType.Sqrt,
                )
                nc.vector.tensor_scalar_max(z2ss[:dt, :1], z2ss[:dt, :1], L2_EPS)
                z2r = sb.tile([P, 1], fp32, tag="z2r")
                nc.vector.reciprocal(z2r[:dt, :1], z2ss[:dt, :1])
                nc.scalar.activation(
                    out=z2r[:dt, :1], in_=z2r[:dt, :1],
                    func=mybir.ActivationFunctionType.Copy, scale=SC,
                )
                nc.vector.tensor_scalar(
                    out=z2col[:dt, t : t + 1],
                    in0=z2t[:dt, 0:1],
                    scalar1=z2r[:dt, :1],
                    scalar2=None,
                    op0=mybir.AluOpType.mult,
                )

            # ---------- P2: aggregation per dest tile ----------
            for t in range(NTILES):
                dt = min(P, NS - t * P)
                gx = gxpool.tile([P, NT, C // 2], i32, tag="gx")
                nc.gpsimd.dma_gather(
                    gx[:], xw_gat[:].bitcast(i32),
                    aggidx_sb[:, t * NT * 8 : (t + 1) * NT * 8],
                    NT * P, NT * P, C // 2, elem_step=C // 2,
                    single_packet=False,
                )
                gxbf = gx[:].bitcast(bf16)
                z1_ps = ps_z1.tile([P, C], fp32, space="PSUM", tag="z1")
                for u in range(NT):
                    col = t * NT + u
                    sel = selpool.tile([P, P], bf16, tag="sel")
                    nc.vector.tensor_scalar(
                        out=sel[:],
                        in0=iota_f[:],
                        scalar1=dloc_sb[:, col : col + 1],
                        scalar2=nrm_sb[:, col : col + 1],
                        op0=mybir.AluOpType.is_equal,
                        op1=mybir.AluOpType.mult,
                    )
                    nc.tensor.matmul(
                        out=z1_ps[:],
                        lhsT=sel[:],
                        rhs=gxbf[:, u, 0:C],
                        start=(u == 0),
                        stop=(u == NT - 1),
                    )
                asm = sb.tile([P, 132], bf16, tag="asm")
                nc.vector.memset(asm[:], 0.0)
                nc.vector.tensor_copy(out=asm[:, 0:C], in_=z1_ps[:])
                nc.vector.tensor_copy(
                    out=asm[:dt, C : C + 1], in_=z2col[:dt, t : t + 1]
                )
                nc.gpsimd.dma_start(
                    out=ag2_in[t * P : t * P + dt, :], in_=asm[:dt, :]
                )

            nc.gpsimd.collective_compute(
                "AllGather",
                mybir.AluOpType.bypass,
                ins=[ag2_in[:]],
                outs=[z1_ext[:]],
                replica_groups=rg,
            )
            nc.sync.dma_start(out=z1_gat[0:N, 0:132], in_=z1_ext[:])

            # ---------- P4: decode ----------
            vf = accpool.tile([P, NCH], fp32, tag="vf")
            vn = accpool.tile([P, NCH], fp32, tag="vn")
            NB = (NT + 1) // 2
            for t in range(NTILES):
                for half in range(2):
                    u0 = half * NB
                    nu = min(NB, NT - u0)
                    if nu <= 0:
                        continue
                    gr = decpool.tile([P, NB, C], i32, tag="gr")
                    nc.gpsimd.dma_gather(
                        gr[:, :nu, :], z1_gat[:].bitcast(i32),
                        aggidx_sb[:, (t * NT + u0) * 8 : (t * NT + u0 + nu) * 8],
                        nu * P, nu * P, C, elem_step=C, single_packet=False,
                    )
                    gc = decpool.tile([P, NB, C], i32, tag="gc")
                    nc.gpsimd.dma_gather(
                        gc[:, :nu, :], z1_gat[:].bitcast(i32),
                        dstidx_sb[:, (t * NT + u0) * 8 : (t * NT + u0 + nu) * 8],
                        nu * P, nu * P, C, elem_step=C, single_packet=False,
                    )
                    grbf = gr[:].bitcast(bf16)
                    gcbf = gc[:].bitcast(bf16)
                    for u in range(nu):
                        col = t * NT + u0 + u
                        prod = sb.tile([P, C], bf16, tag="prod")
                        nc.vector.tensor_tensor(
                            out=prod[:],
                            in0=grbf[:, u, 0:C],
                            in1=gcbf[:, u, 0:C],
                            op=mybir.AluOpType.mult,
                        )
                        nc.vector.reduce_sum(
                            vf[:, col : col + 1], prod[:], axis=mybir.AxisListType.X
                        )
                        nc.vector.tensor_tensor(
                            out=vn[:, col : col + 1],
                            in0=grbf[:, u, C : C + 1],
                            in1=gcbf[:, u, C : C + 1],
                            op=mybir.AluOpType.add,
                        )

            sf = accpool.tile([P, NCH], fp32, tag="sf")
            nc.scalar.activation(
                out=sf[:], in_=vf[:], func=mybir.ActivationFunctionType.Sigmoid
            )
            sn = accpool.tile([P, NCH], fp32, tag="sn")
            nc.scalar.activation(
                out=sn[:], in_=vn[:], func=mybir.ActivationFunctionType.Sigmoid
            )
            t1 = accpool.tile([P, NCH], fp32, tag="t1")
            nc.vector.tensor_tensor(
                out=t1[:], in0=sf[:], in1=sf[:], op=mybir.AluOpType.mult
            )
            t2 = accpool.tile([P, NCH], fp32, tag="t2")
            nc.vector.tensor_tensor(
                out=t2[:], in0=sf[:], in1=sn[:], op=mybir.AluOpType.mult
            )
            t3 = accpool.tile([P, NCH], fp32, tag="t3")
            nc.vector.tensor_tensor(
                out=t3[:], in0=t1[:], in1=sn[:], op=mybir.AluOpType.add
            )
            res = accpool.tile([P, NCH], fp32, tag="res")
            nc.vector.tensor_tensor(
                out=res[:], in0=t3[:], in1=t2[:], op=mybir.AluOpType.subtract
            )
            nc.gpsimd.dma_start(out=dec_out[:], in_=res[:])

    nc.finalize()
    return nc


def _wrap16(logical):
    """logical [n] int -> stored [128, n//16] int16 (wrap by 16, tile x8)."""
    n = logical.shape[0]
    st = logical.reshape(n // 16, 16).T.astype(np.int16)
    return np.tile(st, (8, 1))


def _prepare(x, x2, W2, b2, Wg, bg, W22, edge_index):
    x = np.asarray(x, dtype=_F32)
    x2 = np.asarray(x2, dtype=_F32)
    W2 = np.asarray(W2, dtype=_F32)
    b2 = np.asarray(b2, dtype=_F32)
    Wg = np.asarray(Wg, dtype=_F32)
    bg = np.asarray(bg, dtype=_F32)
    W22 = np.asarray(W22, dtype=_F32)
    row = np.asarray(edge_index[0], dtype=np.int64).astype(np.int32)
    col = np.asarray(edge_index[1], dtype=np.int64).astype(np.int32)

    deg = np.bincount(col, minlength=N).astype(np.float64) + 1.0
    dinv = (1.0 / np.sqrt(deg)).astype(_F32)

    # ---- per-core edge partition / chunking ----
    cores = []
    NT = 0
    for k in range(NCORES):
        lo, hi = k * NS, (k + 1) * NS
        m = (col >= lo) & (col < hi)
        es = row[m]
        ed = col[m]
        en = dinv[es] * dinv[ed]
        orig = np.nonzero(m)[0].astype(np.int64)
        loop = np.arange(lo, hi, dtype=np.int32)
        a_src = np.concatenate([es, loop, np.full(NS, BIAS_ROW, np.int32)])
        a_dst = np.concatenate([ed, loop, loop])
        a_nrm = np.concatenate([en, dinv[loop] * dinv[loop], np.ones(NS, _F32)])
        a_org = np.concatenate([orig, np.full(2 * NS, -1, np.int64)])
        tl = (a_dst - lo) // P
        order = np.argsort(tl, kind="stable")
        a_src, a_dst, a_nrm, a_org, tl = (
            a_src[order], a_dst[order], a_nrm[order], a_org[order], tl[order])
        cnt = np.bincount(tl, minlength=NTILES)
        NT = max(NT, int(np.ceil(cnt.max() / P)))
        cores.append((a_src, a_dst, a_nrm, a_org, cnt))

    NCH = NTILES * NT
    W2T_aug = np.zeros((5 * P, C), _F32)
    W2T_aug[:IN_DIM] = W2.T
    W2T_aug[IN_DIM] = b2
    W22T = np.zeros((KT * P, 2), _BF16)
    W22T[:N] = W22.T.astype(_BF16)
    xw_extra = np.zeros((16, C), _BF16)
    xw_extra[0] = bg.astype(_BF16)
    z1_extra = np.zeros((16, 2 * C), _BF16)

    in_maps = []
    outpos = []   # (orig_ids, p_idx, col_idx) per core
    for k in range(NCORES):
        lo = k * NS
        a_src, a_dst, a_nrm, a_org, cnt = cores[k]
        S = np.full((NTILES, NT * P), ZERO_ROW, np.int32)   # src logical lists
        D = np.full((NTILES, NT * P), ZERO_ROW, np.int32)   # dst logical lists
        DL = np.zeros((P, NCH), _F32)
        NR = np.zeros((P, NCH), _F32)
        off = 0
        oid, opp, occ = [], [], []
        for t in range(NTILES):
            n_t = int(cnt[t])
            sl = slice(off, off + n_t)
            j = np.arange(n_t)
            S[t, :n_t] = a_src[sl]
            D[t, :n_t] = a_dst[sl]
            p = j % P
            u = j // P
            DL[p, t * NT + u] = (a_dst[sl] - lo - t * P).astype(_F32)
            NR[p, t * NT + u] = a_nrm[sl]
            real = a_org[sl] >= 0
            oid.append(a_org[sl][real])
            opp.append(p[real])
            occ.append(t * NT + u[real])
            off += n_t
        outpos.append((np.concatenate(oid), np.concatenate(opp),
                       np.concatenate(occ)))

        agg_idx = np.concatenate([_wrap16(S[t]) for t in range(NTILES)], axis=1)
        dst_idx = np.concatenate([_wrap16(D[t]) for t in range(NTILES)], axis=1)

        xT_aug = np.zeros((5 * P, NS), _F32)
        xT_aug[:IN_DIM] = x[lo : lo + NS].T
        xT_aug[IN_DIM] = 1.0
        x2T = np.zeros((KT * P, NS), _BF16)
        x2T[:N] = x2[lo : lo + NS].T.astype(_BF16)

        in_maps.append({
            "xT_aug": xT_aug,
            "w2T_aug": W2T_aug,
            "wgT": np.ascontiguousarray(Wg.T),
            "x2T": x2T,
            "w22T": W22T,
            "xw_extra": xw_extra,
            "z1_extra": z1_extra,
            "agg_idx": agg_idx,
            "dst_idx": dst_idx,
            "destloc": DL,
            "enorm": NR,
        })

    nc = _build_program(NT)
    return nc, in_maps, outpos


def kernel(x, x2, W2, b2, Wg, bg, W22, edge_index):
    nc, in_maps, outpos = _prepare(x, x2, W2, b2, Wg, bg, W22, edge_index)
    r = run_bass_kernel_spmd(nc, in_maps, list(range(NCORES)))
    global _last_results
    _last_results = r

    out = np.zeros(E, _F32)
    for k in range(NCORES):
        dec = r.results[k]["dec_out"]
        oid, opp, occ = outpos[k]
        out[oid] = dec[opp, occ]
    return out



# revision 10
# speedup vs baseline: 1.4049x; 1.4049x over previous
"""Trainium2 Bass kernel for nn_DVGAE (GCN encoder + inner-product decoder).

v3 strategy (8 NeuronCores, SPMD), all per-core:
  - Edges partitioned by SOURCE core. P0: h = l2norm(x@W2.T+b2)*1.8, xw = h@Wg.T
    computed on the local 1250-node shard (bf16 PE, skinny outputs), xw kept in
    SBUF as fp8 tiles.
  - Aggregation as dense block-matmuls: host builds A[s_local, dest] fp8 blocks
    (norm weights folded, self-loops included); PE computes partial
    z1[10000,128] = A.T @ xw_local with NO gathers. Bias via K=1 matmul.
    Dest slots parity-interleaved so the partial write has 512B runs.
  - ReduceScatter(add) -> z1 local shard [1250,128] bf16 (tiny output, cheap
    collective), then ONE AllGather of (z1 fp8 | z2col bf16) 136B rows written
    strided into a 256B-row gather table.
  - z2 = l2norm(x2@W22.T)*0.8 streamed bf16 (x2 must stay bf16 for accuracy),
    skinny [*,2] matmul orientation -> near-zero PE cost.
  - Decode partitioned by source: local endpoint rows expanded on PE via
    host-built one-hot SelT fp8 matmuls (no local gather); remote endpoint via
    one dma_gather of 256B rows; wide DVE prod; bf16 add-tree reduce; sigmoids.
"""

import sys

sys.path.insert(0, "/opt/trn_rl_repo")

import numpy as np
import ml_dtypes

import concourse.bass as bass
import concourse.bacc as bacc
import concourse.mybir as mybir
import concourse.tile as tile
from concourse.bass_utils import run_bass_kernel_spmd
from concourse.masks import make_identity

P = 128
NCORES = 8
N = 10000
E = 320000
IN_DIM = 512
C = 128
NS = N // NCORES            # 1250 nodes per core
SW = 10                     # local src windows of 128 (last 98)
SPANS = 40                  # 256-node dest spans (last has 16 nodes)
KT = 80                     # z2 k-tiles of 128 (10240 padded)
KB = 4                      # k-tiles per x2 stream DMA
GC = 3                      # decode chunks per PSUM group
SCALING_FACTOR = 1.8
SC = 0.8
L2_EPS = 1e-12
PADN = N                    # remote pad index -> zeroed row
ZROWS = 10112               # z1x rows (79*128, >= N + pad)

bf16 = mybir.dt.bfloat16
fp32 = mybir.dt.float32
fp8 = mybir.dt.float8e4
i16 = mybir.dt.int16
i32 = mybir.dt.int32

_F32 = np.float32
_BF16 = ml_dtypes.bfloat16
_FP8 = ml_dtypes.float8_e4m3


def _build_program(NU):
    """NU = chunks per decode window (compile-time uniform)."""
    NQ = SW * NU             # total decode chunks
    nc = bacc.Bacc(None, target_bir_lowering=False, debug=False)

    # ---------------- I/O ----------------
    xT = nc.declare_dram_parameter("xT", [5 * P, NS], bf16, isOutput=False)
    w2T = nc.declare_dram_parameter("w2T", [5 * P, C], bf16, isOutput=False)
    wgT = nc.declare_dram_parameter("wgT", [C, C], bf16, isOutput=False)
    x2T = nc.declare_dram_parameter("x2T", [KT * P, NS], bf16, isOutput=False)
    w22T = nc.declare_dram_parameter("w22T", [KT * P, 2], bf16, isOutput=False)
    Ablk = nc.declare_dram_parameter("Ablk", [P, SPANS * 2 * SW * P], fp8, isOutput=False)
    bmask = nc.declare_dram_parameter("bmask", [1, SPANS * 2 * P], fp8, isOutput=False)
    bgrow = nc.declare_dram_parameter("bgrow", [1, C], fp32, isOutput=False)
    selT = nc.declare_dram_parameter("selT", [P, NQ * P], fp8, isOutput=False)
    ridx = nc.declare_dram_parameter("ridx", [P, NQ * 8], i16, isOutput=False)

    dec_out = nc.declare_dram_parameter("dec_out", [P, NQ], fp32, isOutput=True)

    # ------------- internal DRAM -------------
    partial = nc.dram_tensor("partial", [N, C], bf16)
    z1loc_d = nc.dram_tensor("z1loc_d", [NS, C], bf16)
    ag_in = nc.dram_tensor("ag_in", [NS, 65], bf16)
    z1x = nc.dram_tensor("z1x", [ZROWS, 128], bf16)

    rg = [list(range(NCORES))]

    with tile.TileContext(nc) as tc:
        with (
            tc.tile_pool(name="const", bufs=1) as cpool,
            tc.tile_pool(name="sb", bufs=3) as sb,
            tc.tile_pool(name="x2s", bufs=2) as x2pool,
            tc.tile_pool(name="ab", bufs=2) as apool,
            tc.tile_pool(name="part", bufs=2) as ppool,
            tc.tile_pool(name="gr", bufs=2) as grpool,
            tc.tile_pool(name="prod", bufs=2) as prpool,
            tc.tile_pool(name="tree", bufs=2) as trpool,
            tc.tile_pool(name="ps_h", bufs=1, space="PSUM") as psH,
            tc.tile_pool(name="ps_t", bufs=1, space="PSUM") as psT,
            tc.tile_pool(name="ps_x", bufs=1, space="PSUM") as psX,
            tc.tile_pool(name="ps_g", bufs=2, space="PSUM") as psG,
            tc.tile_pool(name="ps_z", bufs=1, space="PSUM") as psZ,
            tc.tile_pool(name="ps_e", bufs=2, space="PSUM") as psE,
        ):
            # ---------- constants ----------
            xT_sb = cpool.tile([P, 5, NS], bf16)
            nc.sync.dma_start(out=xT_sb[:], in_=xT[:].rearrange("(t p) n -> p t n", p=P))
            w2T_sb = cpool.tile([P, 5, C], bf16)
            nc.sync.dma_start(out=w2T_sb[:], in_=w2T[:].rearrange("(t p) c -> p t c", p=P))
            wgT_sb = cpool.tile([P, C], bf16)
            nc.sync.dma_start(out=wgT_sb[:], in_=wgT[:])
            w22T_sb = cpool.tile([P, KT, 2], bf16)
            nc.sync.dma_start(out=w22T_sb[:], in_=w22T[:].rearrange("(t p) c -> p t c", p=P))
            ridx_sb = cpool.tile([P, NQ * 8], i16)
            nc.scalar.dma_start(out=ridx_sb[:], in_=ridx[:])
            bm_sb = cpool.tile([1, SPANS * 2 * P], fp8)
            nc.scalar.dma_start(out=bm_sb[:], in_=bmask[:])
            bg_sb = cpool.tile([1, C], fp32)
            nc.scalar.dma_start(out=bg_sb[:], in_=bgrow[:])
            bgb = cpool.tile([1, C], bf16)
            nc.vector.tensor_copy(out=bgb[:], in_=bg_sb[:])

            ident = cpool.tile([P, P], fp32)
            make_identity(nc, ident[:])
            identb = cpool.tile([P, P], bf16)
            nc.vector.tensor_copy(out=identb[:], in_=ident[:])

            xw_tiles = cpool.tile([P, SW, C], fp8)
            nc.vector.memset(xw_tiles[:], 0.0)

            # zero the pad rows of the remote gather table
            zpad = cpool.tile([P, SW, 128], bf16)
            nc.vector.memset(zpad[:], 0.0)
            for zg in range(8):
                za = min(10, 79 - zg * 10)
                nc.gpsimd.dma_start(
                    out=z1x[zg * 1280 : zg * 1280 + za * P, :].rearrange(
                        "(a p) c -> p a c", p=P
                    ),
                    in_=zpad[:, :za, :],
                )

            # ---------- P0: h = l2norm(x@W2.T+b2)*1.8 ; xw = h@Wg.T (fp8) ----------
            for nb in range(SW):
                n0 = nb * P
                nw = min(P, NS - n0)
                h_ps = psH.tile([P, C], fp32, space="PSUM", tag="h")
                for t in range(5):
                    nc.tensor.matmul(
                        out=h_ps[:nw],
                        lhsT=xT_sb[:, t, n0 : n0 + nw],
                        rhs=w2T_sb[:, t, :],
                        start=(t == 0),
                        stop=(t == 4),
                    )
                sq = sb.tile([P, C], fp32, tag="sq")
                ss = sb.tile([P, 1], fp32, tag="ss")
                nc.scalar.activation(
                    out=sq[:nw], in_=h_ps[:nw],
                    func=mybir.ActivationFunctionType.Square,
                    accum_out=ss[:nw, :1],
                )
                nc.scalar.activation(
                    out=ss[:nw, :1], in_=ss[:nw, :1],
                    func=mybir.ActivationFunctionType.Sqrt,
                )
                nc.vector.tensor_scalar_max(ss[:nw, :1], ss[:nw, :1], L2_EPS)
                rinv = sb.tile([P, 1], fp32, tag="rinv")
                nc.vector.reciprocal(rinv[:nw, :1], ss[:nw, :1])
                nc.scalar.activation(
                    out=rinv[:nw, :1], in_=rinv[:nw, :1],
                    func=mybir.ActivationFunctionType.Copy, scale=SCALING_FACTOR,
                )
                h2 = sb.tile([P, C], bf16, tag="h2")
                nc.scalar.activation(
                    out=h2[:nw], in_=h_ps[:nw],
                    func=mybir.ActivationFunctionType.Copy, scale=rinv[:nw, :1],
                )
                h2T_ps = psT.tile([P, P], bf16, space="PSUM", tag="tp")
                nc.tensor.matmul(
                    out=h2T_ps[:, :nw], lhsT=h2[:nw], rhs=identb[:nw, :nw],
                    is_transpose=True,
                )
                h2T = sb.tile([P, P], bf16, tag="h2T")
                nc.vector.tensor_copy(out=h2T[:, :nw], in_=h2T_ps[:, :nw])
                xw_ps = psX.tile([P, C], fp32, space="PSUM", tag="xw")
                nc.tensor.matmul(
                    out=xw_ps[:nw], lhsT=h2T[:, :nw], rhs=wgT_sb[:], start=True, stop=True
                )
                nc.scalar.activation(
                    out=xw_tiles[:nw, nb, :], in_=xw_ps[:nw],
                    func=mybir.ActivationFunctionType.Copy,
                )

            # ---------- aggregation: partial z1 = A.T @ xw (dense blocks) ----------
            ABG = 4     # spans per A DMA
            for g in range(SPANS // ABG):
                at = apool.tile([P, ABG * 2 * SW * P], fp8, tag="at")
                nc.scalar.dma_start(
                    out=at[:],
                    in_=Ablk[:, g * ABG * 2 * SW * P : (g + 1) * ABG * 2 * SW * P],
                )
                for s4 in range(ABG):
                    sp = g * ABG + s4
                    rows = 256 if sp < SPANS - 1 else N - 256 * (SPANS - 1)
                    pp = rows // 2
                    aps = psG.tile([P, 2, C], fp32, space="PSUM", tag="agg")
                    for par in range(2):
                        w = sp * 2 + par
                        for sbk in range(SW):
                            off = ((s4 * 2 + par) * SW + sbk) * P
                            nc.tensor.matmul(
                                out=aps[:, par, :],
                                lhsT=at[:, off : off + P],
                                rhs=xw_tiles[:, sbk, :],
                                start=(sbk == 0),
                                stop=False,
                            )
                        nc.tensor.matmul(
                            out=aps[:, par, :],
                            lhsT=bm_sb[:, w * P : (w + 1) * P],
                            rhs=bgb[:],
                            start=False,
                            stop=True,
                        )
                    spart = ppool.tile([P, 2, C], bf16, tag="sp")
                    nc.scalar.activation(
                        out=spart[:], in_=aps[:],
                        func=mybir.ActivationFunctionType.Copy,
                    )
                    nc.scalar.dma_start(
                        out=partial[sp * 256 : sp * 256 + rows, :].rearrange(
                            "(p s) c -> p s c", s=2
                        ),
                        in_=spart[:pp, :, :],
                    )

            nc.gpsimd.collective_compute(
                "ReduceScatter",
                mybir.AluOpType.add,
                ins=[partial[:]],
                outs=[z1loc_d[:]],
                replica_groups=rg,
            )
            # deprioritized big const: decode Sel table (needed only at decode)
            selT_sb = cpool.tile([P, NQ * P], fp8)
            nc.gpsimd.dma_start(out=selT_sb[:], in_=selT[:])

            # ---------- z2 = l2norm(x2 @ W22.T) * 0.8 (skinny matmuls) ----------
            zacc0 = cpool.tile([P, 2 * SW], fp32)
            zacc1 = cpool.tile([P, 2 * SW], fp32)
            zacc = [zacc0, zacc1]
            for b in range(KT // KB):
                xt = x2pool.tile([P, KB, SW * P], bf16, tag="x2t")
                nc.vector.memset(xt[:, :, NS : SW * P], 0.0)
                nc.sync.dma_start(
                    out=xt[:, :, 0:NS],
                    in_=x2T[b * KB * P : (b + 1) * KB * P, :].rearrange(
                        "(a p) n -> p a n", p=P
                    ),
                )
                zps = psZ.tile([P, 2 * SW], fp32, space="PSUM", tag="z2")
                for ncb in range(SW):
                    n0 = ncb * P
                    for a in range(KB):
                        nc.tensor.matmul(
                            out=zps[:, 2 * ncb : 2 * ncb + 2],
                            lhsT=xt[:, a, n0 : n0 + P],
                            rhs=w22T_sb[:, b * KB + a, :],
                            start=(a == 0),
                            stop=(a == KB - 1),
                        )
                if b == 0:
                    nc.vector.tensor_copy(out=zacc[0][:], in_=zps[:])
                else:
                    nc.vector.tensor_tensor(
                        out=zacc[b % 2][:], in0=zacc[(b - 1) % 2][:], in1=zps[:],
                        op=mybir.AluOpType.add,
                    )
            zfin = zacc[(KT // KB - 1) % 2]

            z2colb = cpool.tile([P, SW, 1], bf16)
            nc.vector.memset(z2colb[:], 0.0)
            for ncb in range(SW):
                nw = min(P, NS - ncb * P)
                z2sq = sb.tile([P, 2], fp32, tag="z2sq")
                z2ss = sb.tile([P, 1], fp32, tag="z2ss")
                nc.scalar.activation(
                    out=z2sq[:nw], in_=zfin[:nw, 2 * ncb : 2 * ncb + 2],
                    func=mybir.ActivationFunctionType.Square,
                    accum_out=z2ss[:nw, :1],
                )
                nc.scalar.activation(
                    out=z2ss[:nw, :1], in_=z2ss[:nw, :1],
                    func=mybir.ActivationFunctionType.Sqrt,
                )
                nc.vector.tensor_scalar_max(z2ss[:nw, :1], z2ss[:nw, :1], L2_EPS)
                z2r = sb.tile([P, 1], fp32, tag="z2r")
                nc.vector.reciprocal(z2r[:nw, :1], z2ss[:nw, :1])
                nc.scalar.activation(
                    out=z2r[:nw, :1], in_=z2r[:nw, :1],
                    func=mybir.ActivationFunctionType.Copy, scale=SC,
                )
                nc.vector.tensor_scalar(
                    out=z2colb[:nw, ncb, :],
                    in0=zfin[:nw, 2 * ncb : 2 * ncb + 1],
                    scalar1=z2r[:nw, :1],
                    scalar2=None,
                    op0=mybir.AluOpType.mult,
                )

            # ---------- z1 local shard -> asm (fp8 z1 | bf16 z2col) ----------
            z1l_bf = cpool.tile([P, SW, C], bf16)
            nc.vector.memset(z1l_bf[:], 0.0)
            nc.sync.dma_start(
                out=z1l_bf[:, 0:9, :],
                in_=z1loc_d[0 : 9 * P, :].rearrange("(w p) c -> p w c", p=P),
            )
            nc.sync.dma_start(out=z1l_bf[0:98, 9, :], in_=z1loc_d[9 * P : NS, :])
            z1l_f8 = cpool.tile([P, SW, C], fp8)
            nc.vector.tensor_copy(out=z1l_f8[:], in_=z1l_bf[:])

            z1f8b = z1l_f8[:].bitcast(bf16)
            nc.gpsimd.dma_start(
                out=ag_in[0 : 9 * P, 0:64].rearrange("(w p) c -> p w c", p=P),
                in_=z1f8b[:, 0:9, :],
            )
            nc.gpsimd.dma_start(out=ag_in[9 * P : NS, 0:64], in_=z1f8b[0:98, 9, :])
            nc.gpsimd.dma_start(
                out=ag_in[0 : 9 * P, 64:65].rearrange("(w p) c -> p w c", p=P),
                in_=z2colb[:, 0:9, :],
            )
            nc.gpsimd.dma_start(out=ag_in[9 * P : NS, 64:65], in_=z2colb[0:98, 9, :])

            nc.gpsimd.collective_compute(
                "AllGather",
                mybir.AluOpType.bypass,
                ins=[ag_in[:]],
                outs=[z1x[0:N, 0:65]],
                replica_groups=rg,
            )

            # ---------- decode ----------
            vf_all = cpool.tile([P, NQ], fp32)
            vn_all = cpool.tile([P, NQ], fp32)
            NGW = (NU + GC - 1) // GC
            for win in range(SW):
                grt = grpool.tile([P, NU, 64], i32, tag="gr")
                nc.gpsimd.dma_gather(
                    grt[:], z1x[:].bitcast(i32),
                    ridx_sb[:, win * NU * 8 : (win + 1) * NU * 8],
                    NU * P, NU * P, 64, elem_step=64,
                    single_packet=False,
                )
                gr8 = grt[:].bitcast(fp8)
                grb = grt[:].bitcast(bf16)
                prodb = prpool.tile([P, NU, C], bf16, tag="pr")
                for grp in range(NGW):
                    u0 = grp * GC
                    ng = min(GC, NU - u0)
                    pE = psE.tile([P, GC, 129], fp32, space="PSUM", tag="E")
                    for u in range(u0, u0 + ng):
                        q = win * NU + u
                        nc.tensor.matmul(
                            out=pE[:, u - u0, 0:128],
                            lhsT=selT_sb[:, q * P : (q + 1) * P],
                            rhs=z1l_f8[:, win, :],
                            start=True, stop=True,
                        )
                        nc.tensor.matmul(
                            out=pE[:, u - u0, 128:129],
                            lhsT=selT_sb[:, q * P : (q + 1) * P],
                            rhs=z2colb[:, win, :],
                            start=True, stop=True,
                        )
                    nc.vector.tensor_tensor(
                        out=prodb[:, u0 : u0 + ng, :],
                        in0=pE[:, 0:ng, 0:128],
                        in1=gr8[:, u0 : u0 + ng, 0:128],
                        op=mybir.AluOpType.mult,
                    )
                    nc.vector.tensor_tensor(
                        out=vn_all[:, win * NU + u0 : win * NU + u0 + ng],
                        in0=pE[:, 0:ng, 128:129],
                        in1=grb[:, u0 : u0 + ng, 64:65],
                        op=mybir.AluOpType.add,
                    )
                # bf16 add-tree reduce over channels (2x DVE)
                trA = trpool.tile([P, NU, 64], bf16, tag="trA")
                trB = trpool.tile([P, NU, 32], bf16, tag="trB")
                nc.vector.tensor_tensor(
                    out=trA[:], in0=prodb[:, :, 0:64], in1=prodb[:, :, 64:128],
                    op=mybir.AluOpType.add,
                )
                nc.vector.tensor_tensor(
                    out=trB[:], in0=trA[:, :, 0:32], in1=trA[:, :, 32:64],
                    op=mybir.AluOpType.add,
                )
                nc.vector.tensor_tensor(
                    out=trA[:, :, 0:16], in0=trB[:, :, 0:16], in1=trB[:, :, 16:32],
                    op=mybir.AluOpType.add,
                )
                nc.vector.tensor_tensor(
                    out=trB[:, :, 0:8], in0=trA[:, :, 0:8], in1=trA[:, :, 8:16],
                    op=mybir.AluOpType.add,
                )
                nc.vector.tensor_tensor(
                    out=trA[:, :, 0:4], in0=trB[:, :, 0:4], in1=trB[:, :, 4:8],
                    op=mybir.AluOpType.add,
                )
                nc.vector.tensor_tensor(
                    out=trB[:, :, 0:2], in0=trA[:, :, 0:2], in1=trA[:, :, 2:4],
                    op=mybir.AluOpType.add,
                )
                nc.vector.tensor_tensor(
                    out=vf_all[:, win * NU : (win + 1) * NU],
                    in0=trB[:, :, 0:1], in1=trB[:, :, 1:2],
                    op=mybir.AluOpType.add,
                )

            sf = cpool.tile([P, NQ], fp32)
            nc.scalar.activation(
                out=sf[:], in_=vf_all[:], func=mybir.ActivationFunctionType.Sigmoid
            )
            sn = cpool.tile([P, NQ], fp32)
            nc.scalar.activation(
                out=sn[:], in_=vn_all[:], func=mybir.ActivationFunctionType.Sigmoid
            )
            t1 = cpool.tile([P, NQ], fp32)
            nc.vector.tensor_tensor(out=t1[:], in0=sf[:], in1=sf[:], op=mybir.AluOpType.mult)
            t2 = cpool.tile([P, NQ], fp32)
            nc.vector.tensor_tensor(out=t2[:], in0=sf[:], in1=sn[:], op=mybir.AluOpType.mult)
            t3 = cpool.tile([P, NQ], fp32)
            nc.vector.tensor_tensor(out=t3[:], in0=t1[:], in1=sn[:], op=mybir.AluOpType.add)
            res = cpool.tile([P, NQ], fp32)
            nc.vector.tensor_tensor(out=res[:], in0=t3[:], in1=t2[:], op=mybir.AluOpType.subtract)
            nc.gpsimd.dma_start(out=dec_out[:], in_=res[:])

    nc.finalize()
    return nc


def _wrap16(logical):
    """logical [n] int (n % 16 == 0) -> [128, n//16] i16 idx table."""
    n = logical.shape[0]
    st = logical.reshape(n // 16, 16).T.astype(np.int16)
    return np.tile(st, (8, 1))


def _prepare(x, x2, W2, b2, Wg, bg, W22, edge_index):
    x = np.asarray(x, dtype=_F32)
    x2 = np.asarray(x2, dtype=_F32)
    W2 = np.asarray(W2, dtype=_F32)
    b2 = np.asarray(b2, dtype=_F32)
    Wg = np.asarray(Wg, dtype=_F32)
    bg = np.asarray(bg, dtype=_F32)
    W22 = np.asarray(W22, dtype=_F32)
    row = np.asarray(edge_index[0], dtype=np.int64).astype(np.int32)
    col = np.asarray(edge_index[1], dtype=np.int64).astype(np.int32)

    deg = np.bincount(col, minlength=N).astype(np.float64) + 1.0
    dinv = (1.0 / np.sqrt(deg)).astype(_F32)

    # shared weight tensors
    w2T = np.zeros((5 * P, C), _BF16)
    w2T[:IN_DIM] = W2.T.astype(_BF16)
    w2T[IN_DIM] = b2.astype(_BF16)
    wgT = np.ascontiguousarray(Wg.T).astype(_BF16)
    w22T = np.zeros((KT * P, 2), _BF16)
    w22T[:N] = W22.T.astype(_BF16)
    bgrow = bg.reshape(1, C)

    # ---- per-core edge partition (by source) ----
    cores = []
    NU = 0
    for k in range(NCORES):
        lo = k * NS
        m = (row >= lo) & (row < lo + NS)
        es = row[m] - lo          # local src
        ed = col[m]               # global dest
        en = (dinv[row[m]] * dinv[col[m]]).astype(_F32)
        orig = np.nonzero(m)[0].astype(np.int64)
        # decode chunking by src window
        win = es // P
        order = np.argsort(win, kind="stable")
        es_s, ed_s, org_s, win_s = es[order], ed[order], orig[order], win[order]
        cnt = np.bincount(win_s, minlength=SW)
        NU = max(NU, int(np.ceil(cnt.max() / P)))
        cores.append((es, ed, en, es_s, ed_s, org_s, cnt, lo))

    NQ = SW * NU
    in_maps = []
    outpos = []
    for k in range(NCORES):
        es, ed, en, es_s, ed_s, org_s, cnt, lo = cores[k]

        # aggregation A blocks: [s, span, par, sb, dslot]
        A = np.zeros((P, SPANS, 2, SW, P), _F32)
        asrc = np.concatenate([es, np.arange(NS, dtype=np.int32)])
        adst = np.concatenate([ed, np.arange(lo, lo + NS, dtype=np.int32)])
        awt = np.concatenate([en, (dinv[lo : lo + NS] ** 2).astype(_F32)])
        sp = adst // 256
        par = adst % 2
        dslot = (adst % 256) // 2
        np.add.at(A, (asrc % P, sp, par, asrc // P, dslot), awt)
        Ahost = A.reshape(P, SPANS * 2 * SW * P).astype(_FP8)

        bm = np.zeros((1, SPANS * 2 * P), _FP8)
        dl = np.arange(lo, lo + NS, dtype=np.int32)
        bm[0, (dl // 256 * 2 + dl % 2) * P + (dl % 256) // 2] = 1.0

        # decode tables
        sel = np.zeros((P, NQ * P), _FP8)
        rlog = np.full(NQ * P, PADN, np.int32)
        oid = np.full(NQ * P, -1, np.int64)
        off = 0
        for w in range(SW):
            n_w = int(cnt[w])
            j = np.arange(n_w)
            qcol = w * NU + j // P
            lane = j % P
            flat = qcol * P + lane
            sel[es_s[off : off + n_w] - w * P, flat] = 1.0
            rlog[flat] = ed_s[off : off + n_w]
            oid[flat] = org_s[off : off + n_w]
            off += n_w
        ridx_t = np.concatenate(
            [_wrap16(rlog[w * NU * P : (w + 1) * NU * P]) for w in range(SW)], axis=1
        )
        real = oid >= 0
        outpos.append((oid[real], (np.arange(NQ * P) % P)[real],
                       (np.arange(NQ * P) // P)[real]))

        xTk = np.zeros((5 * P, NS), _BF16)
        xTk[:IN_DIM] = x[lo : lo + NS].T.astype(_BF16)
        xTk[IN_DIM] = 1.0
        x2Tk = np.zeros((KT * P, NS), _BF16)
        x2Tk[:N] = x2[lo : lo + NS].T.astype(_BF16)

        in_maps.append({
            "xT": xTk,
            "w2T": w2T,
            "wgT": wgT,
            "x2T": x2Tk,
            "w22T": w22T,
            "Ablk": Ahost,
            "bmask": bm,
            "bgrow": bgrow,
            "selT": sel,
            "ridx": ridx_t,
        })

    nc = _build_program(NU)
    return nc, in_maps, outpos


def kernel(x, x2, W2, b2, Wg, bg, W22, edge_index):
    nc, in_maps, outpos = _prepare(x, x2, W2, b2, Wg, bg, W22, edge_index)
    r = run_bass_kernel_spmd(nc, in_maps, list(range(NCORES)))
    global _last_results
    _last_results = r

    out = np.zeros(E, _F32)
    for k in range(NCORES):
        dec = r.results[k]["dec_out"]
        oid, lane, qcol = outpos[k]
        out[oid] = dec[lane, qcol]
    return out
